# revision 16
# baseline (speedup 1.0000x reference)
"""Trainium2 Bass kernel for nn_ClauseInferModule (NSFR clause inference).

Math (per step, per clause c):
  g[b,gi,s,l] = R[c,b, I[c,gi,s,l]]
  p = softand_L(g), r = softor_S(p), R_new = softor_pair(R, r)

With gamma=0.001 the soft ops are within gamma*log(n) <= 0.002 of hard
min/max; replacing every soft op with hard min/max gives a global rel err of
2.6e-3 vs the jax reference on the key-0 inputs (tolerance 2e-2), and the
reference's renormalization `where(m>1, s/m, s)` is a no-op for these inputs
(max m = 0.9999934 < 1). So the kernel computes exactly:

  r[b,gi] = max_s min_l R[b, I[gi,s,l]];  R_new = max(R, r)

Sharding: clause-parallel, 2 clauses per core; partitions = 2*64 = 128
(rows 0-63 clause 2k, rows 64-127 clause 2k+1). Per step, chunked over gi
(128 gi per chunk = 4096 gathered cols): Pool ap_gather (1.39 ns/col is the
critical path) overlapped with a DVE tensor_tensor min/max tree. Chunk column
order is literal-major then subst-major (pos = l*1024 + s*128 + gi) so every
reduction level is a packed half-vs-half tensor_tensor, and the per-chunk
epilogue updates a ping-pong R copy so later chunks still gather from the
old state.
"""

import numpy as np

C, B, G, S, L = 16, 64, 2048, 8, 4
NCORES = 8
CPC = C // NCORES          # clauses per core
P = CPC * B                # 128 partitions
NIDX = G * S * L           # 65536 gathered elements per clause
IDX_COLS = NIDX // 16      # wrapped idx columns per partition (4096)

# gi chunk sizes per step: big chunks amortize the per-gather Q7 launch; the
# tail shrinks gradually (each chunk's DVE chain must fit under the next
# chunk's gather, or DVE backlog stalls the step boundary) so the last DVE
# chain — the serial part of each step boundary — is short. Chunks below 64
# gi are pointless: the gather's cost floor is the 2048-col R input ap size.
CHUNK_SIZES = [256] * 6 + [160, 112, 96, 80, 64]
CHUNKS = []
_st = 0
for _n in CHUNK_SIZES:
    CHUNKS.append((_st, _n))
    _st += _n
assert _st == G
MAXN = max(CHUNK_SIZES)

_nc_cache = {}


def _build(steps: int, debug: bool = False):
    import concourse.bacc as bacc
    import concourse.mybir as mybir
    import concourse.tile as tile

    f32 = mybir.dt.float32
    bf16 = mybir.dt.bfloat16
    i16 = mybir.dt.int16
    ALU = mybir.AluOpType

    nc = bacc.Bacc("TRN2", target_bir_lowering=False, debug=debug)
    xin = nc.dram_tensor("xin", [P, G], f32, kind="ExternalInput")
    idxin = nc.dram_tensor("idxin", [P, IDX_COLS], i16, kind="ExternalInput")
    outd = nc.dram_tensor("outd", [P, G], f32, kind="ExternalOutput")

    idx0_cols = 2 * CHUNK_SIZES[0]  # idx block of the first chunk

    with tile.TileContext(nc) as tc:
        with (
            tc.tile_pool(name="state", bufs=1) as st,
            tc.tile_pool(name="work", bufs=3) as wp,
            tc.tile_pool(name="small", bufs=2) as sp,
        ):
            RA = st.tile([P, G], f32, tag="RA")
            RB = st.tile([P, G], f32, tag="RB")
            IDX = st.tile([P, IDX_COLS], i16, tag="IDX")
            # chunk-0 idx block first so the first gather starts after
            # xin + a small idx transfer (subtile deps gate per region)
            nc.sync.dma_start(out=IDX[:, 0:idx0_cols], in_=idxin.ap()[:, 0:idx0_cols])
            nc.sync.dma_start(out=RA[:], in_=xin.ap())
            nc.sync.dma_start(
                out=IDX[:, idx0_cols:IDX_COLS], in_=idxin.ap()[:, idx0_cols:IDX_COLS]
            )

            cur, nxt = RA, RB
            for t in range(steps):
                ic = 0  # running idx column offset
                for st_gi, n in CHUNKS:
                    cols = 32 * n
                    g = wp.tile([P, 32 * MAXN], f32, tag="g")
                    nc.gpsimd.ap_gather(
                        g[:, 0:cols], cur[:], IDX[:, ic : ic + 2 * n],
                        channels=P, num_elems=G, d=1, num_idxs=cols,
                    )
                    ic += 2 * n
                    # min over L: blocks [L0|L1|L2|L3] of 8n cols each.
                    # t0 onward is bf16: adds <=0.002 abs rounding once per
                    # value (verified 3.9e-3 total vs reference) and gets the
                    # 2x_1p DVE mode for every op below the first level.
                    t0 = sp.tile([P, 8 * MAXN], bf16, tag="t0")
                    t1 = sp.tile([P, 8 * MAXN], bf16, tag="t1")
                    p = sp.tile([P, 8 * MAXN], bf16, tag="p")
                    nc.vector.tensor_tensor(
                        out=t0[:, 0 : 8 * n], in0=g[:, 0 : 8 * n],
                        in1=g[:, 8 * n : 16 * n], op=ALU.min,
                    )
                    nc.vector.tensor_tensor(
                        out=t1[:, 0 : 8 * n], in0=g[:, 16 * n : 24 * n],
                        in1=g[:, 24 * n : 32 * n], op=ALU.min,
                    )
                    nc.vector.tensor_tensor(
                        out=p[:, 0 : 8 * n], in0=t0[:, 0 : 8 * n],
                        in1=t1[:, 0 : 8 * n], op=ALU.min,
                    )
                    # max over S: p layout pos = s*n + gi, halve 3 times
                    u = sp.tile([P, 4 * MAXN], bf16, tag="u")
                    v = sp.tile([P, 2 * MAXN], bf16, tag="v")
                    w = sp.tile([P, MAXN], bf16, tag="w")
                    nc.vector.tensor_tensor(
                        out=u[:, 0 : 4 * n], in0=p[:, 0 : 4 * n],
                        in1=p[:, 4 * n : 8 * n], op=ALU.max,
                    )
                    nc.vector.tensor_tensor(
                        out=v[:, 0 : 2 * n], in0=u[:, 0 : 2 * n],
                        in1=u[:, 2 * n : 4 * n], op=ALU.max,
                    )
                    nc.vector.tensor_tensor(
                        out=w[:, 0:n], in0=v[:, 0:n], in1=v[:, n : 2 * n], op=ALU.max,
                    )
                    # per-chunk pairwise-softor epilogue into the ping-pong copy
                    nc.vector.tensor_tensor(
                        out=nxt[:, st_gi : st_gi + n],
                        in0=cur[:, st_gi : st_gi + n],
                        in1=w[:, 0:n],
                        op=ALU.max,
                    )
                cur, nxt = nxt, cur

            # split output DMA: earlier pieces only depend on the chunks
            # covering their columns and overlap the tail of the last step;
            # the final piece is small so the post-epilogue transfer is short
            cuts = [0, 1536, CHUNKS[-1][0], G]
            for a, b in zip(cuts[:-1], cuts[1:]):
                nc.sync.dma_start(out=outd.ap()[:, a:b], in_=cur[:, a:b])

    nc.compile()
    return nc


def _pack_idx(I_cl: np.ndarray) -> np.ndarray:
    """Clause index tensor (G, S, L) -> (16, IDX_COLS) int16 wrapped layout.

    Per n-gi chunk the gather's output column pos = l*8n + s*n + gi
    (literal-major blocks so every DVE reduction level is packed halves);
    flat position j of a chunk lives at (partition j%16, column j//16) of the
    chunk's 2n-column idx block."""
    out = np.empty((16, IDX_COLS), dtype=np.int16)
    ic = 0
    for st_gi, n in CHUNKS:
        blk = I_cl[st_gi : st_gi + n]                          # (n, S, L)
        flat = blk.transpose(2, 1, 0).reshape(-1).astype(np.int16)
        out[:, ic : ic + 2 * n] = flat.reshape(2 * n, 16).T
        ic += 2 * n
    return out


def _make_inputs(x: np.ndarray, I: np.ndarray):
    xin = np.concatenate([x, x], axis=0).astype(np.float32)  # (128, G), same all cores
    in_maps = []
    for core in range(NCORES):
        idx_full = np.zeros((P, IDX_COLS), dtype=np.int16)
        for cl_local in range(CPC):
            w = _pack_idx(I[core * CPC + cl_local])  # (16, IDX_COLS)
            for grp in range(4):
                rows = slice(cl_local * 64 + grp * 16, cl_local * 64 + (grp + 1) * 16)
                idx_full[rows] = w
        in_maps.append({"xin": xin, "idxin": idx_full})
    return in_maps


def kernel(x: np.ndarray, I: np.ndarray, infer_step) -> np.ndarray:
    from concourse import bass_utils

    steps = int(infer_step)
    x = np.asarray(x, dtype=np.float32)
    I = np.asarray(I, dtype=np.int32)
    if steps not in _nc_cache:
        _nc_cache[steps] = _build(steps)
    nc = _nc_cache[steps]

    in_maps = _make_inputs(x, I)
    res = bass_utils.run_bass_kernel_spmd(nc, in_maps, list(range(NCORES)))
    out = np.empty((C, B, G), dtype=np.float32)
    for core in range(NCORES):
        o = res.results[core]["outd"]
        out[core * CPC] = o[:64]
        out[core * CPC + 1] = o[64:]
    return out


if __name__ == "__main__":
    x = np.load("/root/problem/x.npy")
    I = np.load("/root/problem/I.npy")
    out = kernel(x, I, 3)
